# revision 13
# baseline (speedup 1.0000x reference)
"""Trainium2 Bass kernel for nn_Fields: 16 per-field MLPs (3->16->16->3, ReLU)
applied to 1M points, sharded over 8 NeuronCores along the point axis.

Dataflow per core (points sharded N/8 per core, padded to 248 chunks of 512):
  DMA x (+ones row) -> L1 row-tiled matmuls (K=4) -> PSUM f32
  -> ReLU evac (ScalarE/VectorE) -> SBUF bf16 h1
  -> L2 block-diag matmuls (K=128) -> PSUM -> ReLU+b2 evac -> SBUF bf16 h2
  -> L3 col-tiled matmuls (M=24) -> PSUM -> Identity+b3 evac -> SBUF f32
  -> DMA out.
b1 is folded into L1 via a ones row appended to x. Biases b2/b3 ride the
PSUM->SBUF evacuation ops as per-partition bias vectors.
"""

from contextlib import ExitStack

import ml_dtypes
import numpy as np

import concourse.bass as bass
import concourse.mybir as mybir
import concourse.tile as tile
from concourse import bacc
from concourse.bass_utils import run_bass_kernel_spmd

N_CORES = 8
NFIELDS = 16
HID = 16
C = 512  # chunk = one fp32 PSUM bank of matmul output
PAIR = 2 * C  # 1024 points
GROUP_PAIRS = 4
GROUP = GROUP_PAIRS * PAIR  # 4096 points

BF16 = mybir.dt.bfloat16
F32 = mybir.dt.float32
AF = mybir.ActivationFunctionType
ALU = mybir.AluOpType

_cache = {}


def build(n_pad, iters=1, evac_mode="balanced"):
    """Build the per-core Bass program for n_pad points (multiple of GROUP)."""
    assert n_pad % GROUP == 0
    n_groups = n_pad // GROUP

    nc = bacc.Bacc(None, target_bir_lowering=False)
    xq = nc.declare_dram_parameter("xq", [4, n_pad], BF16, isOutput=False)
    w1s_d = nc.declare_dram_parameter("w1s", [4, 256], BF16, isOutput=False)
    w2s_d = nc.declare_dram_parameter("w2s", [128, 256], BF16, isOutput=False)
    w3s_d = nc.declare_dram_parameter("w3s", [128, 48], BF16, isOutput=False)
    b2v_d = nc.declare_dram_parameter("b2v", [128, 2], F32, isOutput=False)
    b3v_d = nc.declare_dram_parameter("b3v", [128, 1], F32, isOutput=False)
    y = nc.declare_dram_parameter("y", [128, n_pad // 2], F32, isOutput=True)

    with ExitStack() as ctx:
        tc = ctx.enter_context(tile.TileContext(nc))
        consts = ctx.enter_context(tc.tile_pool(name="consts", bufs=1))
        xpool = ctx.enter_context(tc.tile_pool(name="xpool", bufs=2))
        h1pool = ctx.enter_context(tc.tile_pool(name="h1pool", bufs=12))
        h2pool = ctx.enter_context(tc.tile_pool(name="h2pool", bufs=12))
        opool = ctx.enter_context(tc.tile_pool(name="opool", bufs=2))
        pspool = ctx.enter_context(tc.tile_pool(name="pspool", bufs=4, space="PSUM"))
        
        w1s = consts.tile([4, 256], BF16)
        nc.sync.dma_start(out=w1s, in_=w1s_d[:, :])
        w2s = consts.tile([128, 256], BF16)
        nc.sync.dma_start(out=w2s, in_=w2s_d[:, :])
        w3s = consts.tile([128, 48], BF16)
        nc.sync.dma_start(out=w3s, in_=w3s_d[:, :])
        b2v = consts.tile([128, 2], F32)
        nc.sync.dma_start(out=b2v, in_=b2v_d[:, :])
        b3v = consts.tile([128, 1], F32)
        nc.sync.dma_start(out=b3v, in_=b3v_d[:, :])

        # Greedy engine balancing for PSUM->SBUF evacuation ops: ScalarE
        # (Activation) runs 1 elem/cyc @1.2GHz with 172cyc PSUM const; DVE
        # 1 elem/cyc @0.96GHz with 120cyc const. Assign each op to the
        # engine with lower accumulated busy time.
        load = {"act": 0.0, "dve": 0.0}

        def evac(out_ap, in_ap, fd, bias=None, relu=True):
            # HW-calibrated (A/B on silicon): ScalarE PSUM-src activations run
            # ~2.3x slower than the architectural table; DVE is 1 elem/cyc.
            cost_act = (352 + 2.3 * fd) / 1.2
            cost_dve = (120 + fd) / 0.96
            if evac_mode == "act":
                cost_dve = 1e18
            elif evac_mode == "dve":
                cost_act = 1e18
            if load["act"] + cost_act <= load["dve"] + cost_dve:
                load["act"] += cost_act
                if relu:
                    nc.scalar.activation(out_ap, in_ap, AF.Relu,
                                         bias=0.0 if bias is None else bias)
                else:
                    nc.scalar.activation(out_ap, in_ap, AF.Identity,
                                         bias=0.0 if bias is None else bias)
            else:
                load["dve"] += cost_dve
                if relu:
                    if bias is None:
                        nc.vector.tensor_scalar_max(out_ap, in_ap, 0.0)
                    else:
                        nc.vector.tensor_scalar(out_ap, in_ap, bias, 0.0,
                                                ALU.add, ALU.max)
                else:
                    if bias is None:
                        nc.vector.tensor_copy(out_ap, in_ap)
                    else:
                        nc.vector.tensor_scalar_add(out_ap, in_ap, bias)

        NCH = 2 * GROUP_PAIRS  # chunks per slab/group

        def body(_=None):
            for g in range(n_groups):
                gc = g * GROUP
                xsb = xpool.tile([4, GROUP], BF16)
                nc.gpsimd.dma_start(out=xsb[0:4, :],
                                    in_=xq[0:4, gc : gc + GROUP])
                outsb = opool.tile([128, GROUP_PAIRS * C], F32)

                # ---- phase L1: all chunks of the slab ----
                h1sb = []
                h1ps = []
                for c in range(NCH):
                    h1ps.append(pspool.tile([128, 2 * C], F32, tag="hps", name=f"h1ps_{g}_{c}"))
                    for half in range(2):
                        nc.tensor.matmul(
                            h1ps[c][:, half * C : half * C + C],
                            w1s[0:4, 128 * half : 128 * half + 128],
                            xsb[0:4, c * C : c * C + C],
                            start=True, stop=True,
                        )
                    t = h1pool.tile([128, 2 * C], BF16, tag="h1sb",
                                    name=f"h1sb_{g}_{c}")
                    evac(t, h1ps[c], 2 * C)
                    h1sb.append(t)

                # ---- phase L2: all half-a matmuls, then all half-b ----
                h2A, h2B = [], []
                for p in range(GROUP_PAIRS):
                    t = pspool.tile([128, 2 * C], F32, tag="hps",
                                    name=f"h2a_{g}_{p}")
                    nc.tensor.matmul(t[:, 0:C], w2s[:, 0:128],
                                     h1sb[2 * p][:, 0:C], start=True, stop=True)
                    nc.tensor.matmul(t[:, C : 2 * C], w2s[:, 0:128],
                                     h1sb[2 * p + 1][:, 0:C],
                                     start=True, stop=True)
                    h2A.append(t)
                h2Asb, h2Bsb = [], []
                for p in range(GROUP_PAIRS):
                    t = h2pool.tile([128, 2 * C], BF16, tag="h2sb",
                                    name=f"h2asb_{g}_{p}")
                    evac(t, h2A[p], 2 * C, bias=b2v[:, 0:1])
                    h2Asb.append(t)
                for p in range(GROUP_PAIRS):
                    t = pspool.tile([128, 2 * C], F32, tag="hps",
                                    name=f"h2b_{g}_{p}")
                    nc.tensor.matmul(t[:, 0:C], w2s[:, 128:256],
                                     h1sb[2 * p][:, C : 2 * C],
                                     start=True, stop=True)
                    nc.tensor.matmul(t[:, C : 2 * C], w2s[:, 128:256],
                                     h1sb[2 * p + 1][:, C : 2 * C],
                                     start=True, stop=True)
                    h2B.append(t)
                for p in range(GROUP_PAIRS):
                    t = h2pool.tile([128, 2 * C], BF16, tag="h2sb",
                                    name=f"h2bsb_{g}_{p}")
                    evac(t, h2B[p], 2 * C, bias=b2v[:, 1:2])
                    h2Bsb.append(t)

                # ---- phase L3: 4 col-tiled matmuls per pair into one bank ----
                for p in range(GROUP_PAIRS):
                    oph = pspool.tile([128, C], F32, tag="hps", name=f"oph_{g}_{p}")
                    nc.tensor.matmul(oph[0:24, :], w3s[:, 0:24],
                                     h2Asb[p][:, 0:C], start=True, stop=True,
                                     tile_position=(0, 0))
                    nc.tensor.matmul(oph[64:88, :], w3s[:, 0:24],
                                     h2Asb[p][:, C : 2 * C], start=True,
                                     stop=True, tile_position=(0, 64))
                    nc.tensor.matmul(oph[32:56, :], w3s[:, 24:48],
                                     h2Bsb[p][:, 0:C], start=True, stop=True,
                                     tile_position=(0, 32))
                    nc.tensor.matmul(oph[96:120, :], w3s[:, 24:48],
                                     h2Bsb[p][:, C : 2 * C], start=True,
                                     stop=True, tile_position=(0, 96))
                    evac(outsb[0:120, p * C : p * C + C], oph[0:120, :], C,
                         bias=b3v[0:120, 0:1], relu=False)
                # out DMA: whole quarter-structured slab block, gap rows too
                nc.sync.dma_start(
                    out=y[0:120, g * 2048 : g * 2048 + GROUP_PAIRS * C],
                    in_=outsb[0:120, :])

        if iters == 1:
            body()
        else:
            with tc.For_i(0, iters, 1):
                body()
    nc.finalize()
    return nc


def prep_weights(W1, b1, W2, b2, W3, b3):
    W1 = np.asarray(W1, np.float32); b1 = np.asarray(b1, np.float32)
    W2 = np.asarray(W2, np.float32); b2 = np.asarray(b2, np.float32)
    W3 = np.asarray(W3, np.float32); b3 = np.asarray(b3, np.float32)
    w1s = np.zeros((4, 256), np.float32)
    for half in range(2):
        fb = 8 * half
        for fl in range(8):
            for h in range(HID):
                w1s[0:3, 128 * half + 16 * fl + h] = W1[fb + fl, h, :]
                w1s[3, 128 * half + 16 * fl + h] = b1[fb + fl, h]
    w2s = np.zeros((128, 256), np.float32)
    for half in range(2):
        fb = 8 * half
        for fl in range(8):
            blk = W2[fb + fl]  # [g2, h]
            w2s[16 * fl : 16 * fl + 16,
                128 * half + 16 * fl : 128 * half + 16 * fl + 16] = blk.T
    w3s = np.zeros((128, 48), np.float32)
    for half in range(2):
        fb = 8 * half
        for fl in range(8):
            blk = W3[fb + fl]  # [o, h]
            w3s[16 * fl : 16 * fl + 16,
                24 * half + 3 * fl : 24 * half + 3 * fl + 3] = blk.T
    b2v = np.zeros((128, 2), np.float32)
    for half in range(2):
        b2v[:, half] = b2[8 * half : 8 * half + 8].reshape(128)
    b3v = np.zeros((128, 1), np.float32)
    for q in range(4):
        fb = 8 * (q % 2)
        b3v[32 * q : 32 * q + 24, 0] = b3[fb : fb + 8].reshape(24)
    bf = ml_dtypes.bfloat16
    return {
        "w1s": w1s.astype(bf), "w2s": w2s.astype(bf), "w3s": w3s.astype(bf),
        "b2v": b2v, "b3v": b3v,
    }


def _get_nc(n_pad, iters=1, evac_mode="balanced"):
    key = (n_pad, iters, evac_mode)
    if key not in _cache:
        _cache[key] = build(n_pad, iters, evac_mode)
    return _cache[key]


def run(x_np, weights, n_pad, iters=1, n=None):
    """x_np: [3, N] f32 full; returns [16, 3, N] f32."""
    if n is None:
        n = x_np.shape[1]
    assert n % N_CORES == 0
    npc = n // N_CORES
    assert npc <= n_pad
    nc = _get_nc(n_pad, iters)
    bf = ml_dtypes.bfloat16
    in_maps = []
    for c in range(N_CORES):
        xs = np.zeros((4, n_pad), np.float32)
        xs[0:3, :npc] = x_np[:, c * npc : (c + 1) * npc]
        xs[3, :] = 1.0
        in_maps.append({"xq": xs.astype(bf), **weights})
    res = run_bass_kernel_spmd(nc, in_maps, core_ids=list(range(N_CORES)))
    out = np.empty((NFIELDS, 3, n), np.float32)
    nsl = n_pad // GROUP
    for c in range(N_CORES):
        yc = res.results[c]["y"]  # [128, n_pad//2] quarter-structured
        yv = yc.reshape(128, nsl, GROUP_PAIRS, C)
        oc = np.empty((NFIELDS, 3, nsl, GROUP_PAIRS, 2, C), np.float32)
        for q in range(4):
            blk = yv[32 * q : 32 * q + 24].reshape(8, 3, nsl, GROUP_PAIRS, C)
            oc[8 * (q % 2) : 8 * (q % 2) + 8, :, :, :, q // 2, :] = blk
        out[:, :, c * npc : (c + 1) * npc] = \
            oc.reshape(NFIELDS, 3, n_pad)[:, :, :npc]
    return out


def kernel(x, W1, b1, W2, b2, W3, b3, D):
    x = np.asarray(x, np.float32)
    n = x.shape[2]
    npc = n // N_CORES
    n_pad = ((npc + GROUP - 1) // GROUP) * GROUP
    weights = prep_weights(W1, b1, W2, b2, W3, b3)
    return run(x[0], weights, n_pad)
